# revision 64
# baseline (speedup 1.0000x reference)
"""Trainium2 Bass kernel for nn_Caps_Layer (capsule routing layer).

Reference computation (per batch b of 1024):
  u_hat[b] = (x[b] @ W).reshape(512, 5, 5) -> [5cap(i), 512(j), 5dim(k)]
  4 rounds of routing:
    c = softmax_i(blog); S[i,k] = sum_j c[i,j] u[i,j,k];
    o = S/||S||; blog[i,j] = sum_k o[i,k] u[i,j,k]
  output: o [1024, 5, 5]

Sharding: pure data parallel over batch across 8 cores (128 batches/core).

Per-core design (token-position on SBUF partitions; s = 4p + q):
  - x streamed in 8-batch gpsimd *casting* DMAs (f32 HBM -> fp16 SBUF,
    1920B contiguous runs): halves DMA time (~44us) and makes the whole
    downstream pipeline fp16.
  - PE transposes fp16 tiles at 1 cyc/row (the fp16 identity is the
    moving operand) [128t,120d] -> [120,128] fp16 PSUM (1 bank); PSUM->
    SBUF copies split DVE (fp16 2x mode) / ACT (f32-bitcast halves
    elems); GPSIMD cannot touch PSUM (walrus rule). fp16 GEMM vs fp16 W
    (1 cyc/row) -> u PSUM f32; W stored column-order (k,i) so the
    PSUM->u2 scatter merges (k,i) into one strided ACT copy per chunk.
  - u2 layout [128p, (k5, i5, q4, b)] fp16. Routing fully fp16 (1.5e-3
    global rel err vs 2e-2 budget); every DVE tensor op keeps innermost
    stride-1 fp16 APs to hit the 2x DVE mode. S-sums via PE ones-matmul
    (column sums replicated across partitions), q-accumulated in PSUM.
    k/i-plane sums use paired-plane tree adds (3 insts), or single
    X-axis reduces for the small tail groups (shorter chains).
  - 1/sqrt(ss) = exp(-0.5*ln(ss+eps)) on ACT; with the activation-table
    choice pinned to 'natural_log_exp_and_others' (index 6), Copy/Exp/Ln
    all share one table -> a single table load (saves ~47us of reloads).
  - uo products split DVE/Pool; fin on Pool -- balancing the elementwise
    engines (DVE is the pacing engine at ~78% occupancy).
  - Batches in groups of decreasing size [32,32,24,16,16,8]: early
    groups route while later x still streams; it0 S-matmuls are emitted
    per 16-batch routing chunk so they stream during phase 1; each
    routing iteration is emitted as three events (softmax / matmuls /
    squash) ordered by a tuned static time estimate so the in-order
    engine queues don't head-of-line block.
"""

import numpy as np

NCORES = 8
B, S, D = 1024, 512, 120
NCAP, DCAP = 5, 5
IK = NCAP * DCAP  # 25
BC = B // NCORES  # 128 batches per core
TOK = BC * S
EPS = 1e-7
ROUTINGS = 4

QB = 4                      # s-phases per partition (s = 4p + q)
CB = 8                      # batches per DMA chunk
NCHUNK = BC // CB           # 16
GROUPS = [32, 32, 24, 16, 16, 8]
CHUNK_RT = 16               # batches per routing psum chunk
UO_ON_DVE = {0, 5}          # groups whose uo-mul runs on DVE (rest: Pool)
XT_DVE_CHUNKS = 4           # first chunks: xt copies all on (idle) DVE
XT_MIX_CHUNKS = 4           # next chunks: alternate DVE/ACT
SCHED_CHUNK_T = 2.8         # est. us between chunk completions
SCHED_D0 = (0.10, 3.0)      # it0 duration estimate (a*bg + b)
SCHED_D1 = (0.18, 5.0)      # it1..3 duration estimate
SCHED_FR = (0.55, 0.30)     # sx/mm shares of an iter (sq gets the rest)
SCHED_IT0 = (0.4, 1.2)      # it0: mm offset after chunk; sq lag
ACT_TABLE_ID = 6            # natural_log_exp_and_others: copy+exp+ln

# Measured routing-iter completion times (us, from TimelineSim feedback);
# None -> use the analytic estimate. Keys: (g, it).
ITER_TIMES = None
EMIT_LOG = []               # [(tag, inst_lo, inst_hi)] filled during _build


def _patch_act_tables():
    """Pin the act-table chooser to set 6 (has copy+exp+ln together).

    bacc's insert_act_table_loads picks the FIRST table containing each
    activation's func, which alternates exp->set0 / ln->set5 and inserts a
    1283ns table load per switch. Presenting sets 0..5 as empty (names and
    positions preserved, so act_func_set_id stays a canonical act_info.json
    index) makes every func resolve to set 6 -> one load total.
    """
    import concourse.bacc as bacc

    if getattr(bacc, "_caps_tables_patched", False):
        return
    orig = bacc.get_activation_tables

    def patched(arch):
        tables = dict(orig(arch))
        names = list(tables.keys())
        out = {}
        for i, name in enumerate(names):
            out[name] = tables[name] if i == ACT_TABLE_ID else set()
        return out

    bacc.get_activation_tables = patched
    bacc._caps_tables_patched = True


def _build():
    import concourse.bass as bass
    import concourse.bacc as bacc
    import concourse.tile as tile
    from concourse import mybir
    from concourse.masks import make_identity

    _patch_act_tables()

    f32 = mybir.dt.float32
    f16 = mybir.dt.float16
    AF = mybir.ActivationFunctionType

    nc = bacc.Bacc("TRN2", target_bir_lowering=False, debug=False)
    x_d = nc.dram_tensor("x", [TOK, D], f32, kind="ExternalInput")
    w_d = nc.dram_tensor("w", [D, IK], f32, kind="ExternalInput")
    out_d = nc.dram_tensor("out", [1, BC * IK], f32, kind="ExternalOutput")

    # chunk c, partition p, free (bb, q, d); inner run (q d) = 480 f32
    # contiguous in HBM. token t = (c*CB + bb)*512 + 4p + q.
    x_all = x_d[:, :]

    def xv(c):
        return bass.AP(
            tensor=x_all.tensor,
            offset=x_all.offset + c * CB * S * D,
            ap=[[QB * D, 128], [S * D, CB], [1, QB * D]],
        )

    def ap_of(tile_ap, free_dims, extra_off=0, npart=None):
        part = list(tile_ap.ap[0])
        if npart is not None:
            part = [part[0], npart]
        return bass.AP(
            tensor=tile_ap.tensor,
            offset=tile_ap.offset + extra_off,
            ap=[part] + [list(d) for d in free_dims],
        )

    goff = [sum(GROUPS[:g]) for g in range(len(GROUPS))]

    with tile.TileContext(nc) as tc:
        with (
            tc.tile_pool(name="const", bufs=1) as const,
            tc.tile_pool(name="big", bufs=1) as big,
            tc.tile_pool(name="xin", bufs=12) as xin,
            tc.tile_pool(name="xtsb", bufs=4) as xtsb,
            tc.tile_pool(name="xtps", bufs=4, space="PSUM") as xtps,
            tc.tile_pool(name="ups", bufs=2, space="PSUM") as ups,
            tc.tile_pool(name="ops", bufs=2, space="PSUM") as ops_pool,
        ):
            # ---- constants ----
            w32 = const.tile([128, IK], f32)
            nc.sync.dma_start(out=w32[:D, :], in_=w_d[:, :])
            # w_sb column order (k, i): w_sb[:, k*5+i] = W[:, i*5+k]
            w_sb = const.tile([128, IK], f16)
            w_dst = ap_of(w_sb[:D, :], [[1, NCAP], [NCAP, DCAP]])
            w_src = ap_of(w32[:D, :], [[DCAP, NCAP], [1, DCAP]])
            nc.gpsimd.tensor_copy(out=w_dst, in_=w_src)
            ident = const.tile([128, 128], f16)
            make_identity(nc, ident[:])
            ones16 = const.tile([128, 128], f16)
            nc.vector.memset(ones16[:], 1.0)
            eps_t = const.tile([128, 1], f32)
            nc.vector.memset(eps_t[:], EPS)

            # ---- per-group persistent tensors (fp16 routing state) ----
            # u2/cu layout per group: (k5, i5, q4, b); strides b:1, q:Bg,
            # i:4Bg, k:20Bg
            u2 = [big.tile([128, DCAP, NCAP, QB, bg], f16, name=f"u2_{g}")
                  for g, bg in enumerate(GROUPS)]
            cu = [big.tile([128, DCAP, NCAP, QB, bg], f16, name=f"cu_{g}")
                  for g, bg in enumerate(GROUPS)]   # shared uo/cu scratch
            o2 = [big.tile([128, DCAP, NCAP, bg], f16, name=f"o2_{g}")
                  for g, bg in enumerate(GROUPS)]   # (k, i, b)
            blog = [big.tile([128, NCAP, QB, bg], f16, name=f"blog_{g}")
                    for g, bg in enumerate(GROUPS)]  # (i, q, b)
            scr1 = [big.tile([128, NCAP, QB, bg], f16, name=f"scr1_{g}")
                    for g, bg in enumerate(GROUPS)]
            e_t = [big.tile([128, NCAP, QB, bg], f16, name=f"e_{g}")
                   for g, bg in enumerate(GROUPS)]
            c_t = [big.tile([128, NCAP, QB, bg], f16, name=f"c_{g}")
                   for g, bg in enumerate(GROUPS)]
            z_t = [big.tile([128, QB, bg], f16, name=f"z_{g}")
                   for g, bg in enumerate(GROUPS)]  # (q, b)
            zi_t = [big.tile([128, QB, bg], f16, name=f"zi_{g}")
                    for g, bg in enumerate(GROUPS)]
            sq_t = [big.tile([128, DCAP, NCAP, bg], f16, name=f"sq_{g}")
                    for g, bg in enumerate(GROUPS)]  # (k, i, b)
            ss_t = [big.tile([128, NCAP, bg], f16, name=f"ss_{g}")
                    for g, bg in enumerate(GROUPS)]  # (i, b)
            ssl = [big.tile([128, NCAP, bg], f16, name=f"ssl_{g}")
                   for g, bg in enumerate(GROUPS)]   # ln(ss+eps)
            r_t = [big.tile([128, NCAP, bg], f16, name=f"r_{g}")
                   for g, bg in enumerate(GROUPS)]
            fin = [big.tile([1, bg * IK], f32, name=f"fin_{g}")
                   for g, bg in enumerate(GROUPS)]

            # ================= Phase 1: one 4-batch chunk =================
            def emit_chunk(g, c):
                """DMA 8 batches, transpose, GEMM, scatter into u2[g]."""
                bg = GROUPS[g]
                b_in_g = c * CB - goff[g]  # first batch idx within group
                x4 = xin.tile([128, CB * QB * D], f16, name="x4")
                nc.gpsimd.dma_start(out=x4[:], in_=xv(c))  # casting DMA
                for half in range(2):
                    u_ps = ups.tile([128, 4 * QB * IK], f32, name="u_ps")
                    for hh in range(2):
                        xt_ps = xtps.tile([128, 1024], f16, name="xt_ps")
                        xt_sb = xtsb.tile([128, 1024], f16, name="xt_sb")
                        for bb in range(2):
                            boff = (half * 2 + hh) * 2 + bb
                            for q in range(QB):
                                t = bb * QB + q
                                src = x4[:, boff * QB * D + q * D:
                                         boff * QB * D + (q + 1) * D]
                                nc.tensor.transpose(
                                    xt_ps[:D, t * 128:(t + 1) * 128], src,
                                    ident[:],
                                )
                        # xt copy PSUM->SBUF: GPSIMD can't read PSUM, so
                        # only ACT (f32-bitcast halves elems) or DVE
                        # (fp16 2x mode) are legal. Early chunks go to DVE
                        # (idle until routing ramps up), later ones to ACT.
                        if c < XT_DVE_CHUNKS:
                            dve_copy = True
                        elif c < XT_DVE_CHUNKS + XT_MIX_CHUNKS:
                            dve_copy = (2 * half + hh) % 2 == 0
                        else:
                            dve_copy = False
                        if dve_copy:
                            nc.vector.tensor_copy(out=xt_sb[:D, :],
                                                  in_=xt_ps[:D, :])
                        else:
                            nc.scalar.copy(out=xt_sb[:D, :].bitcast(f32),
                                           in_=xt_ps[:D, :].bitcast(f32))
                        for t in range(8):
                            tt = hh * 8 + t
                            nc.tensor.matmul(
                                u_ps[:, tt * IK:(tt + 1) * IK],
                                xt_sb[:D, t * 128:(t + 1) * 128],
                                w_sb[:D, :],
                                start=True, stop=True,
                            )
                    # scatter psum (hh,bb,q,(k,i)) -> u2[g] (k,i,q,b): one
                    # copy; (k,i) merges on dst (k stride 20bg = 5 * 4bg)
                    src = ap_of(u_ps[:], [[100, 4], [25, 4], [1, 25]])
                    dst = ap_of(u2[g][:], [[1, 4], [bg, 4], [4 * bg, 25]],
                                extra_off=b_in_g + half * 4)
                    nc.scalar.copy(out=dst, in_=src)

            # ================= Phase 2: routing =================
            def emit_sx(g, it):
                """Softmax part of iter it>=1: uo -> blog -> exp -> c -> cu."""
                bg = GROUPS[g]
                u2g, cug, o2g = u2[g][:], cu[g][:], o2[g][:]
                small = bg <= 16   # short-chain variant for tail groups
                PL = 20 * bg   # (i,q,b) plane elems
                IPL = 4 * bg   # (q,b) plane elems
                # uo = u2 * o2_bcast(q); iter ((k,i),(q),(b))
                o2_bc = ap_of(o2g, [[bg, IK], [0, QB], [1, bg]])
                u2_v = ap_of(u2g, [[4 * bg, IK], [bg, QB], [1, bg]])
                cu_v = ap_of(cug, [[4 * bg, IK], [bg, QB], [1, bg]])
                eng = nc.vector if g in UO_ON_DVE else nc.gpsimd
                eng.tensor_mul(cu_v, u2_v, o2_bc)
                bl, s1 = blog[g][:], scr1[g][:]
                if small:
                    # blog_raw = sum_k uo in one X-reduce over k
                    uo_r = ap_of(cug, [[1, PL], [PL, DCAP]])
                    with nc.allow_low_precision(reason="fp16 5-term sum"):
                        nc.vector.reduce_sum(
                            bl, uo_r, axis=mybir.AxisListType.X)
                else:
                    # blog_raw = sum_k uo; paired-plane adds (3 insts):
                    # uo[{0,1}] <- uo[{0,2}] + uo[{1,3}] in one op (safe:
                    # writes trail reads in element order), then combine.
                    uo_k = lambda k: ap_of(cug, [[1, PL]],
                                           extra_off=k * PL)
                    pair_a = ap_of(cug, [[2 * PL, 2], [1, PL]])
                    pair_b = ap_of(cug, [[2 * PL, 2], [1, PL]],
                                   extra_off=PL)
                    s_pairs = ap_of(cug, [[1, 2 * PL]])
                    nc.vector.tensor_add(s_pairs, pair_a, pair_b)
                    nc.vector.tensor_add(bl, uo_k(0), uo_k(1))
                    nc.vector.tensor_add(bl, bl, uo_k(4))
                # temperature: blog *= r_bcast(q)  (r from prev iter)
                r_bc = ap_of(r_t[g][:], [[bg, NCAP], [0, QB], [1, bg]])
                nc.vector.tensor_mul(bl, bl, r_bc)
                # softmax over i
                nc.scalar.activation(out=e_t[g][:], in_=bl, func=AF.Exp)
                zt, zit = z_t[g][:], zi_t[g][:]
                if small:
                    # z = sum_i e in one X-reduce over i (outer-stride)
                    e_r = ap_of(e_t[g][:], [[1, IPL], [IPL, NCAP]])
                    with nc.allow_low_precision(reason="fp16 5-term sum"):
                        nc.vector.reduce_sum(
                            zt, e_r, axis=mybir.AxisListType.X)
                else:
                    # z tree with one paired-plane add into scr1
                    e_i = lambda i: ap_of(e_t[g][:], [[1, IPL]],
                                          extra_off=i * IPL)
                    zp_a = ap_of(e_t[g][:], [[2 * IPL, 2], [1, IPL]])
                    zp_b = ap_of(e_t[g][:], [[2 * IPL, 2], [1, IPL]],
                                 extra_off=IPL)
                    zp_out = ap_of(s1, [[1, 2 * IPL]])
                    nc.vector.tensor_add(zp_out, zp_a, zp_b)
                    nc.vector.tensor_add(zt, ap_of(s1, [[1, IPL]]),
                                         ap_of(s1, [[1, IPL]],
                                               extra_off=IPL))
                    nc.vector.tensor_add(zt, zt, e_i(4))
                with nc.allow_low_precision(reason="fp16 ok at 2e-2 tol"):
                    nc.vector.reciprocal(zit, zt)
                zi_bc = ap_of(zit, [[0, NCAP], [1, IPL]])
                nc.vector.tensor_mul(c_t[g][:], e_t[g][:], zi_bc)
                # cu = u2 * c_bcast(k)
                c_bc = ap_of(c_t[g][:], [[0, DCAP], [1, PL]])
                nc.vector.tensor_mul(cug, u2g, c_bc)

            def emit_mm(g, it, ci):
                """S-matmuls + o2 copy for one 16-batch routing chunk."""
                bg = GROUPS[g]
                KPL = NCAP * bg
                src_t = u2[g][:] if it == 0 else cu[g][:]
                b0 = ci * CHUNK_RT
                cb = min(CHUNK_RT, bg - b0)
                o_ps = ops_pool.tile([128, CHUNK_RT * IK], f32, name="o_ps")
                for q in range(QB):
                    rhs = ap_of(
                        src_t, [[1, cb], [4 * bg, NCAP], [20 * bg, DCAP]],
                        extra_off=q * bg + b0)
                    nc.tensor.matmul(
                        o_ps[:, :cb * IK], ones16[:], rhs,
                        start=(q == 0), stop=(q == QB - 1),
                    )
                # psum (b,i,k) -> o2 (k,i,b), fp16 cast [ACT]
                dst = ap_of(o2[g][:], [[1, cb], [bg, NCAP], [KPL, DCAP]],
                            extra_off=b0)
                nc.scalar.copy(out=dst, in_=o_ps[:, :cb * IK])

            def emit_sq(g, it):
                """Squash stats: r = exp(-0.5*ln(ss+eps)); ss = sum_k o2^2."""
                bg = GROUPS[g]
                o2g = o2[g][:]
                small = bg <= 16
                KPL = NCAP * bg
                last = it == ROUTINGS - 1
                sqg, ssg = sq_t[g][:], ss_t[g][:]
                nc.vector.tensor_mul(sqg, o2g, o2g)
                if small:
                    sq_r = ap_of(sqg, [[1, KPL], [KPL, DCAP]])
                    with nc.allow_low_precision(reason="fp16 5-term sum"):
                        nc.vector.reduce_sum(
                            ssg, sq_r, axis=mybir.AxisListType.X)
                else:
                    # ss tree with one paired-plane add into sq[{0,1}]
                    sq_k = lambda k: ap_of(sqg, [[1, KPL]], extra_off=k * KPL)
                    sp_a = ap_of(sqg, [[2 * KPL, 2], [1, KPL]])
                    sp_b = ap_of(sqg, [[2 * KPL, 2], [1, KPL]],
                                 extra_off=KPL)
                    sp_out = ap_of(sqg, [[1, 2 * KPL]])
                    nc.vector.tensor_add(sp_out, sp_a, sp_b)
                    nc.vector.tensor_add(ssg, sq_k(0), sq_k(1))
                    nc.vector.tensor_add(ssg, ssg, sq_k(4))
                nc.scalar.activation(
                    out=ssl[g][:], in_=ssg, func=AF.Ln, bias=eps_t[:])
                nc.scalar.activation(
                    out=r_t[g][:], in_=ssl[g][:], func=AF.Exp, scale=-0.5)
                if last:
                    # fin[(b,i,k)] = o2[(k,i,b)] * r, row 0 (rows equal), f32
                    o2_row = ap_of(o2g, [[1, bg], [bg, NCAP], [KPL, DCAP]],
                                   npart=1)
                    r_row = ap_of(r_t[g][:], [[1, bg], [bg, NCAP], [0, DCAP]],
                                  npart=1)
                    fin_v = ap_of(fin[g][:], [[IK, bg], [DCAP, NCAP], [1, DCAP]])
                    nc.gpsimd.tensor_mul(fin_v, o2_row, r_row)
                    nc.sync.dma_start(
                        out=out_d[:, goff[g] * IK:(goff[g] + bg) * IK],
                        in_=fin[g][:],
                    )

            # ---- static emission schedule (time-ordered) ----
            CHUNK_T = SCHED_CHUNK_T
            events = []
            for g, bg in enumerate(GROUPS):
                c0 = goff[g] // CB
                nch = bg // CB
                for j in range(nch):
                    events.append(((c0 + j + 1) * CHUNK_T - 1e-6, 0,
                                   ("chunk", g, c0 + j)))
            for g, bg in enumerate(GROUPS):
                nck = (bg + CHUNK_RT - 1) // CHUNK_RT
                # it0 S-matmul chunks stream as their phase chunks land
                t = 0.0
                for ci in range(nck):
                    hi_b = goff[g] + min((ci + 1) * CHUNK_RT, bg)
                    cc = (hi_b + CB - 1) // CB   # covering phase chunk count
                    t = cc * CHUNK_T + SCHED_IT0[0]
                    events.append((t, 1, ("mm", g, 0, ci)))
                t += SCHED_IT0[1] + 0.01 * bg
                events.append((t, 1, ("sq", g, 0)))
                for it in range(1, ROUTINGS):
                    fr_sq = 1.0 - SCHED_FR[0] - SCHED_FR[1]
                    dur_sx = SCHED_D1[0] * bg * SCHED_FR[0] + SCHED_D1[1] * 0.5
                    dur_mm = SCHED_D1[0] * bg * SCHED_FR[1] + SCHED_D1[1] * 0.25
                    dur_sq = SCHED_D1[0] * bg * fr_sq + SCHED_D1[1] * 0.25
                    if ITER_TIMES and (g, it) in ITER_TIMES:
                        t = ITER_TIMES[(g, it)]
                    events.append((t, 1, ("sx", g, it)))
                    for ci in range(nck):
                        events.append((t + dur_sx + dur_mm * (ci + 1) / nck,
                                       1, ("mm", g, it, ci)))
                    t += dur_sx + dur_mm + dur_sq
                    events.append((t, 1, ("sq", g, it)))
            events.sort(key=lambda ev: (ev[0], ev[1]))

            def next_inst_id():
                # consume one id as a marker; returns its numeric value
                return int(nc.get_next_instruction_name().split("-")[-1])

            EMIT_LOG.clear()
            for _, _, ev in events:
                lo = next_inst_id()
                if ev[0] == "chunk":
                    emit_chunk(ev[1], ev[2])
                elif ev[0] == "mm":
                    emit_mm(ev[1], ev[2], ev[3])
                elif ev[0] == "sx":
                    emit_sx(ev[1], ev[2])
                else:
                    emit_sq(ev[1], ev[2])
                EMIT_LOG.append((ev, lo, next_inst_id()))
    nc.compile()
    return nc


_NC = None


def kernel(x: np.ndarray, W: np.ndarray) -> np.ndarray:
    from concourse.bass_utils import run_bass_kernel_spmd

    global _NC
    if _NC is None:
        _NC = _build()

    x = np.ascontiguousarray(x, dtype=np.float32)
    w = np.ascontiguousarray(W.reshape(D, IK), dtype=np.float32)
    xs = x.reshape(NCORES, TOK, D)
    in_maps = [{"x": xs[i], "w": w} for i in range(NCORES)]
    res = run_bass_kernel_spmd(_NC, in_maps, core_ids=list(range(NCORES)))
    out = np.concatenate(
        [r["out"].reshape(BC, NCAP, DCAP) for r in res.results], axis=0
    )
    return out


if __name__ == "__main__":
    rng = np.random.default_rng(0)
    x = rng.standard_normal((B, S, D), dtype=np.float32)
    W = rng.standard_normal((1, D, IK), dtype=np.float32) * 0.1
    out = kernel(x, W)
    print(out.shape, out.dtype)


# revision 68
# speedup vs baseline: 1.0015x; 1.0015x over previous
"""Trainium2 Bass kernel for nn_Caps_Layer (capsule routing layer).

Reference computation (per batch b of 1024):
  u_hat[b] = (x[b] @ W).reshape(512, 5, 5) -> [5cap(i), 512(j), 5dim(k)]
  4 rounds of routing:
    c = softmax_i(blog); S[i,k] = sum_j c[i,j] u[i,j,k];
    o = S/||S||; blog[i,j] = sum_k o[i,k] u[i,j,k]
  output: o [1024, 5, 5]

Sharding: pure data parallel over batch across 8 cores (128 batches/core).

Per-core design (token-position on SBUF partitions; s = 4p + q):
  - x streamed in 8-batch gpsimd *casting* DMAs (f32 HBM -> fp16 SBUF,
    1920B contiguous runs): halves DMA time (~44us) and makes the whole
    downstream pipeline fp16.
  - PE transposes fp16 tiles at 1 cyc/row (the fp16 identity is the
    moving operand) [128t,120d] -> [120,128] fp16 PSUM (1 bank); PSUM->
    SBUF copies split DVE (fp16 2x mode) / ACT (f32-bitcast halves
    elems); GPSIMD cannot touch PSUM (walrus rule). fp16 GEMM vs fp16 W
    (1 cyc/row) -> u PSUM f32; W stored column-order (k,i) so the
    PSUM->u2 scatter merges (k,i) into one strided ACT copy per chunk.
  - u2 layout [128p, (k5, i5, q4, b)] fp16. Routing fully fp16 (1.5e-3
    global rel err vs 2e-2 budget); every DVE tensor op keeps innermost
    stride-1 fp16 APs to hit the 2x DVE mode. S-sums via PE ones-matmul
    (column sums replicated across partitions), q-accumulated in PSUM.
    k/i-plane sums use paired-plane tree adds (3 insts), or single
    X-axis reduces for the small tail groups (shorter chains).
  - 1/sqrt(ss) = exp(-0.5*ln(ss+eps)) on ACT; with the activation-table
    choice pinned to 'natural_log_exp_and_others' (index 6), Copy/Exp/Ln
    all share one table -> a single table load (saves ~47us of reloads).
  - uo products split DVE/Pool; fin on Pool -- balancing the elementwise
    engines (DVE is the pacing engine at ~78% occupancy).
  - Batches in groups of decreasing size [32,32,24,16,16,8]: early
    groups route while later x still streams; it0 S-matmuls are emitted
    per 16-batch routing chunk so they stream during phase 1; each
    routing iteration is emitted as three events (softmax / matmuls /
    squash) ordered by a tuned static time estimate so the in-order
    engine queues don't head-of-line block.
"""

import numpy as np

NCORES = 8
B, S, D = 1024, 512, 120
NCAP, DCAP = 5, 5
IK = NCAP * DCAP  # 25
BC = B // NCORES  # 128 batches per core
TOK = BC * S
EPS = 1e-7
ROUTINGS = 4

QB = 4                      # s-phases per partition (s = 4p + q)
CB = 8                      # batches per DMA chunk
NCHUNK = BC // CB           # 16
GROUPS = [32, 32, 24, 16, 16, 8]
CHUNK_RT = 16               # batches per routing psum chunk
UO_ON_DVE = {0, 5}          # groups whose uo-mul runs on DVE (rest: Pool)
XT_DVE_CHUNKS = 4           # first chunks: xt copies all on (idle) DVE
XT_MIX_CHUNKS = 4           # next chunks: alternate DVE/ACT
SCHED_CHUNK_T = 2.8         # est. us between chunk completions
SCHED_D0 = (0.10, 3.0)      # it0 duration estimate (a*bg + b)
SCHED_D1 = (0.18, 5.0)      # it1..3 duration estimate
SCHED_FR = (0.55, 0.30)     # sx/mm shares of an iter (sq gets the rest)
SCHED_IT0 = (0.4, 1.2)      # it0: mm offset after chunk; sq lag
ACT_TABLE_ID = 6            # natural_log_exp_and_others: copy+exp+ln

# Measured routing-iter completion times (us, from TimelineSim feedback);
# None -> use the analytic estimate. Keys: (g, it).
ITER_TIMES = None
EMIT_LOG = []               # [(tag, inst_lo, inst_hi)] filled during _build


def _patch_act_tables():
    """Pin the act-table chooser to set 6 (has copy+exp+ln together).

    bacc's insert_act_table_loads picks the FIRST table containing each
    activation's func, which alternates exp->set0 / ln->set5 and inserts a
    1283ns table load per switch. Presenting sets 0..5 as empty (names and
    positions preserved, so act_func_set_id stays a canonical act_info.json
    index) makes every func resolve to set 6 -> one load total.
    """
    import concourse.bacc as bacc

    if getattr(bacc, "_caps_tables_patched", False):
        return
    orig = bacc.get_activation_tables

    def patched(arch):
        tables = dict(orig(arch))
        names = list(tables.keys())
        out = {}
        for i, name in enumerate(names):
            out[name] = tables[name] if i == ACT_TABLE_ID else set()
        return out

    bacc.get_activation_tables = patched
    bacc._caps_tables_patched = True


def _build():
    import concourse.bass as bass
    import concourse.bacc as bacc
    import concourse.tile as tile
    from concourse import mybir
    from concourse.masks import make_identity

    _patch_act_tables()

    f32 = mybir.dt.float32
    f16 = mybir.dt.float16
    AF = mybir.ActivationFunctionType

    nc = bacc.Bacc("TRN2", target_bir_lowering=False, debug=False)
    x_d = nc.dram_tensor("x", [TOK, D], f32, kind="ExternalInput")
    w_d = nc.dram_tensor("w", [D, IK], f32, kind="ExternalInput")
    out_d = nc.dram_tensor("out", [1, BC * IK], f32, kind="ExternalOutput")

    # chunk c, partition p, free (bb, q, d); inner run (q d) = 480 f32
    # contiguous in HBM. token t = (c*CB + bb)*512 + 4p + q.
    x_all = x_d[:, :]

    def xv(c):
        return bass.AP(
            tensor=x_all.tensor,
            offset=x_all.offset + c * CB * S * D,
            ap=[[QB * D, 128], [S * D, CB], [1, QB * D]],
        )

    def ap_of(tile_ap, free_dims, extra_off=0, npart=None):
        part = list(tile_ap.ap[0])
        if npart is not None:
            part = [part[0], npart]
        return bass.AP(
            tensor=tile_ap.tensor,
            offset=tile_ap.offset + extra_off,
            ap=[part] + [list(d) for d in free_dims],
        )

    goff = [sum(GROUPS[:g]) for g in range(len(GROUPS))]

    with tile.TileContext(nc) as tc:
        with (
            tc.tile_pool(name="const", bufs=1) as const,
            tc.tile_pool(name="big", bufs=1) as big,
            tc.tile_pool(name="xin", bufs=12) as xin,
            tc.tile_pool(name="xtsb", bufs=4) as xtsb,
            tc.tile_pool(name="xtps", bufs=4, space="PSUM") as xtps,
            tc.tile_pool(name="ups", bufs=2, space="PSUM") as ups,
            tc.tile_pool(name="ops", bufs=2, space="PSUM") as ops_pool,
        ):
            # ---- constants ----
            w32 = const.tile([128, IK], f32)
            nc.sync.dma_start(out=w32[:D, :], in_=w_d[:, :])
            # w_sb column order (k, i): w_sb[:, k*5+i] = W[:, i*5+k]
            w_sb = const.tile([128, IK], f16)
            w_dst = ap_of(w_sb[:D, :], [[1, NCAP], [NCAP, DCAP]])
            w_src = ap_of(w32[:D, :], [[DCAP, NCAP], [1, DCAP]])
            nc.gpsimd.tensor_copy(out=w_dst, in_=w_src)
            ident = const.tile([128, 128], f16)
            make_identity(nc, ident[:])
            ones16 = const.tile([128, 128], f16)
            nc.vector.memset(ones16[:], 1.0)
            eps_t = const.tile([128, 1], f32)
            nc.vector.memset(eps_t[:], EPS)

            # ---- per-group persistent tensors (fp16 routing state) ----
            # u2/cu layout per group: (k5, i5, q4, b); strides b:1, q:Bg,
            # i:4Bg, k:20Bg
            u2 = [big.tile([128, DCAP, NCAP, QB, bg], f16, name=f"u2_{g}")
                  for g, bg in enumerate(GROUPS)]
            cu = [big.tile([128, DCAP, NCAP, QB, bg], f16, name=f"cu_{g}")
                  for g, bg in enumerate(GROUPS)]   # shared uo/cu scratch
            o2 = [big.tile([128, DCAP, NCAP, bg], f16, name=f"o2_{g}")
                  for g, bg in enumerate(GROUPS)]   # (k, i, b)
            blog = [big.tile([128, NCAP, QB, bg], f16, name=f"blog_{g}")
                    for g, bg in enumerate(GROUPS)]  # (i, q, b)
            scr1 = [big.tile([128, NCAP, QB, bg], f16, name=f"scr1_{g}")
                    for g, bg in enumerate(GROUPS)]
            e_t = [big.tile([128, NCAP, QB, bg], f16, name=f"e_{g}")
                   for g, bg in enumerate(GROUPS)]
            c_t = [big.tile([128, NCAP, QB, bg], f16, name=f"c_{g}")
                   for g, bg in enumerate(GROUPS)]
            z_t = [big.tile([128, QB, bg], f16, name=f"z_{g}")
                   for g, bg in enumerate(GROUPS)]  # (q, b)
            zi_t = [big.tile([128, QB, bg], f16, name=f"zi_{g}")
                    for g, bg in enumerate(GROUPS)]
            sq_t = [big.tile([128, DCAP, NCAP, bg], f16, name=f"sq_{g}")
                    for g, bg in enumerate(GROUPS)]  # (k, i, b)
            ss_t = [big.tile([128, NCAP, bg], f16, name=f"ss_{g}")
                    for g, bg in enumerate(GROUPS)]  # (i, b)
            ssl = [big.tile([128, NCAP, bg], f16, name=f"ssl_{g}")
                   for g, bg in enumerate(GROUPS)]   # ln(ss+eps)
            r_t = [big.tile([128, NCAP, bg], f16, name=f"r_{g}")
                   for g, bg in enumerate(GROUPS)]
            fin = [big.tile([1, bg * IK], f32, name=f"fin_{g}")
                   for g, bg in enumerate(GROUPS)]

            # ================= Phase 1: one 4-batch chunk =================
            def emit_chunk(g, c):
                """DMA 8 batches, transpose, GEMM, scatter into u2[g]."""
                bg = GROUPS[g]
                b_in_g = c * CB - goff[g]  # first batch idx within group
                x4 = xin.tile([128, CB * QB * D], f16, name="x4")
                nc.gpsimd.dma_start(out=x4[:], in_=xv(c))  # casting DMA
                for half in range(2):
                    u_ps = ups.tile([128, 4 * QB * IK], f32, name="u_ps")
                    for hh in range(2):
                        xt_ps = xtps.tile([128, 1024], f16, name="xt_ps")
                        xt_sb = xtsb.tile([128, 1024], f16, name="xt_sb")
                        for bb in range(2):
                            boff = (half * 2 + hh) * 2 + bb
                            for q in range(QB):
                                t = bb * QB + q
                                src = x4[:, boff * QB * D + q * D:
                                         boff * QB * D + (q + 1) * D]
                                nc.tensor.transpose(
                                    xt_ps[:D, t * 128:(t + 1) * 128], src,
                                    ident[:],
                                )
                        # xt copy PSUM->SBUF: GPSIMD can't read PSUM, so
                        # only ACT (f32-bitcast halves elems) or DVE
                        # (fp16 2x mode) are legal. Early chunks go to DVE
                        # (idle until routing ramps up), later ones to ACT.
                        if c < XT_DVE_CHUNKS:
                            dve_copy = True
                        elif c < XT_DVE_CHUNKS + XT_MIX_CHUNKS:
                            dve_copy = (2 * half + hh) % 2 == 0
                        else:
                            dve_copy = False
                        if dve_copy:
                            nc.vector.tensor_copy(out=xt_sb[:D, :],
                                                  in_=xt_ps[:D, :])
                        else:
                            nc.scalar.copy(out=xt_sb[:D, :].bitcast(f32),
                                           in_=xt_ps[:D, :].bitcast(f32))
                        for t in range(8):
                            tt = hh * 8 + t
                            nc.tensor.matmul(
                                u_ps[:, tt * IK:(tt + 1) * IK],
                                xt_sb[:D, t * 128:(t + 1) * 128],
                                w_sb[:D, :],
                                start=True, stop=True,
                            )
                    # scatter psum (hh,bb,q,(k,i)) -> u2[g] (k,i,q,b): one
                    # copy; (k,i) merges on dst (k stride 20bg = 5 * 4bg)
                    src = ap_of(u_ps[:], [[100, 4], [25, 4], [1, 25]])
                    dst = ap_of(u2[g][:], [[1, 4], [bg, 4], [4 * bg, 25]],
                                extra_off=b_in_g + half * 4)
                    nc.scalar.copy(out=dst, in_=src)

            # ================= Phase 2: routing =================
            def emit_sx(g, it):
                """Softmax part of iter it>=1: uo -> blog -> exp -> c -> cu."""
                bg = GROUPS[g]
                u2g, cug, o2g = u2[g][:], cu[g][:], o2[g][:]
                small = bg <= 16   # short-chain variant for tail groups
                PL = 20 * bg   # (i,q,b) plane elems
                IPL = 4 * bg   # (q,b) plane elems
                # uo = u2 * o2_bcast(q); iter ((k,i),(q),(b))
                o2_bc = ap_of(o2g, [[bg, IK], [0, QB], [1, bg]])
                u2_v = ap_of(u2g, [[4 * bg, IK], [bg, QB], [1, bg]])
                cu_v = ap_of(cug, [[4 * bg, IK], [bg, QB], [1, bg]])
                eng = nc.vector if g in UO_ON_DVE else nc.gpsimd
                eng.tensor_mul(cu_v, u2_v, o2_bc)
                bl, s1 = blog[g][:], scr1[g][:]
                if small:
                    # blog_raw = sum_k uo in one X-reduce over k
                    uo_r = ap_of(cug, [[1, PL], [PL, DCAP]])
                    with nc.allow_low_precision(reason="fp16 5-term sum"):
                        nc.vector.reduce_sum(
                            bl, uo_r, axis=mybir.AxisListType.X)
                else:
                    # blog_raw = sum_k uo; paired-plane adds (3 insts):
                    # uo[{0,1}] <- uo[{0,2}] + uo[{1,3}] in one op (safe:
                    # writes trail reads in element order), then combine.
                    uo_k = lambda k: ap_of(cug, [[1, PL]],
                                           extra_off=k * PL)
                    pair_a = ap_of(cug, [[2 * PL, 2], [1, PL]])
                    pair_b = ap_of(cug, [[2 * PL, 2], [1, PL]],
                                   extra_off=PL)
                    s_pairs = ap_of(cug, [[1, 2 * PL]])
                    nc.vector.tensor_add(s_pairs, pair_a, pair_b)
                    nc.vector.tensor_add(bl, uo_k(0), uo_k(1))
                    nc.vector.tensor_add(bl, bl, uo_k(4))
                # temperature: blog *= r_bcast(q)  (r from prev iter)
                r_bc = ap_of(r_t[g][:], [[bg, NCAP], [0, QB], [1, bg]])
                nc.vector.tensor_mul(bl, bl, r_bc)
                # softmax over i
                nc.scalar.activation(out=e_t[g][:], in_=bl, func=AF.Exp)
                zt, zit = z_t[g][:], zi_t[g][:]
                if small:
                    # z = sum_i e in one X-reduce over i (outer-stride)
                    e_r = ap_of(e_t[g][:], [[1, IPL], [IPL, NCAP]])
                    with nc.allow_low_precision(reason="fp16 5-term sum"):
                        nc.vector.reduce_sum(
                            zt, e_r, axis=mybir.AxisListType.X)
                else:
                    # z tree with one paired-plane add into scr1
                    e_i = lambda i: ap_of(e_t[g][:], [[1, IPL]],
                                          extra_off=i * IPL)
                    zp_a = ap_of(e_t[g][:], [[2 * IPL, 2], [1, IPL]])
                    zp_b = ap_of(e_t[g][:], [[2 * IPL, 2], [1, IPL]],
                                 extra_off=IPL)
                    zp_out = ap_of(s1, [[1, 2 * IPL]])
                    nc.vector.tensor_add(zp_out, zp_a, zp_b)
                    nc.vector.tensor_add(zt, ap_of(s1, [[1, IPL]]),
                                         ap_of(s1, [[1, IPL]],
                                               extra_off=IPL))
                    nc.vector.tensor_add(zt, zt, e_i(4))
                with nc.allow_low_precision(reason="fp16 ok at 2e-2 tol"):
                    nc.vector.reciprocal(zit, zt)
                zi_bc = ap_of(zit, [[0, NCAP], [1, IPL]])
                nc.vector.tensor_mul(c_t[g][:], e_t[g][:], zi_bc)
                # cu = u2 * c_bcast(k)
                c_bc = ap_of(c_t[g][:], [[0, DCAP], [1, PL]])
                nc.vector.tensor_mul(cug, u2g, c_bc)

            def emit_mm(g, it, ci):
                """S-matmuls + o2 copy for one 16-batch routing chunk."""
                bg = GROUPS[g]
                KPL = NCAP * bg
                src_t = u2[g][:] if it == 0 else cu[g][:]
                b0 = ci * CHUNK_RT
                cb = min(CHUNK_RT, bg - b0)
                o_ps = ops_pool.tile([128, CHUNK_RT * IK], f32, name="o_ps")
                for q in range(QB):
                    rhs = ap_of(
                        src_t, [[1, cb], [4 * bg, NCAP], [20 * bg, DCAP]],
                        extra_off=q * bg + b0)
                    nc.tensor.matmul(
                        o_ps[:, :cb * IK], ones16[:], rhs,
                        start=(q == 0), stop=(q == QB - 1),
                    )
                # psum (b,i,k) -> o2 (k,i,b), fp16 cast [ACT]
                dst = ap_of(o2[g][:], [[1, cb], [bg, NCAP], [KPL, DCAP]],
                            extra_off=b0)
                nc.scalar.copy(out=dst, in_=o_ps[:, :cb * IK])

            def emit_sq(g, it):
                """Squash stats: r = exp(-0.5*ln(ss+eps)); ss = sum_k o2^2."""
                bg = GROUPS[g]
                o2g = o2[g][:]
                small = bg <= 16
                KPL = NCAP * bg
                last = it == ROUTINGS - 1
                sqg, ssg = sq_t[g][:], ss_t[g][:]
                nc.vector.tensor_mul(sqg, o2g, o2g)
                if small:
                    sq_r = ap_of(sqg, [[1, KPL], [KPL, DCAP]])
                    with nc.allow_low_precision(reason="fp16 5-term sum"):
                        nc.vector.reduce_sum(
                            ssg, sq_r, axis=mybir.AxisListType.X)
                else:
                    # ss tree with one paired-plane add into sq[{0,1}]
                    sq_k = lambda k: ap_of(sqg, [[1, KPL]], extra_off=k * KPL)
                    sp_a = ap_of(sqg, [[2 * KPL, 2], [1, KPL]])
                    sp_b = ap_of(sqg, [[2 * KPL, 2], [1, KPL]],
                                 extra_off=KPL)
                    sp_out = ap_of(sqg, [[1, 2 * KPL]])
                    nc.vector.tensor_add(sp_out, sp_a, sp_b)
                    nc.vector.tensor_add(ssg, sq_k(0), sq_k(1))
                    nc.vector.tensor_add(ssg, ssg, sq_k(4))
                nc.scalar.activation(
                    out=ssl[g][:], in_=ssg, func=AF.Ln, bias=eps_t[:])
                nc.scalar.activation(
                    out=r_t[g][:], in_=ssl[g][:], func=AF.Exp, scale=-0.5)
                if last:
                    # fin[(b,i,k)] = o2[(k,i,b)] * r, row 0 (rows equal), f32
                    o2_row = ap_of(o2g, [[1, bg], [bg, NCAP], [KPL, DCAP]],
                                   npart=1)
                    r_row = ap_of(r_t[g][:], [[1, bg], [bg, NCAP], [0, DCAP]],
                                  npart=1)
                    fin_v = ap_of(fin[g][:], [[IK, bg], [DCAP, NCAP], [1, DCAP]])
                    # tail groups: fin on DVE (faster + no Pool queue hop
                    # at the very end of the terminal chain)
                    fin_eng = nc.vector if bg <= CHUNK_RT else nc.gpsimd
                    fin_eng.tensor_mul(fin_v, o2_row, r_row)
                    nc.sync.dma_start(
                        out=out_d[:, goff[g] * IK:(goff[g] + bg) * IK],
                        in_=fin[g][:],
                    )

            # ---- static emission schedule (time-ordered) ----
            CHUNK_T = SCHED_CHUNK_T
            events = []
            for g, bg in enumerate(GROUPS):
                c0 = goff[g] // CB
                nch = bg // CB
                for j in range(nch):
                    events.append(((c0 + j + 1) * CHUNK_T - 1e-6, 0,
                                   ("chunk", g, c0 + j)))
            for g, bg in enumerate(GROUPS):
                nck = (bg + CHUNK_RT - 1) // CHUNK_RT
                # it0 S-matmul chunks stream as their phase chunks land
                t = 0.0
                for ci in range(nck):
                    hi_b = goff[g] + min((ci + 1) * CHUNK_RT, bg)
                    cc = (hi_b + CB - 1) // CB   # covering phase chunk count
                    t = cc * CHUNK_T + SCHED_IT0[0]
                    events.append((t, 1, ("mm", g, 0, ci)))
                t += SCHED_IT0[1] + 0.01 * bg
                events.append((t, 1, ("sq", g, 0)))
                for it in range(1, ROUTINGS):
                    fr_sq = 1.0 - SCHED_FR[0] - SCHED_FR[1]
                    dur_sx = SCHED_D1[0] * bg * SCHED_FR[0] + SCHED_D1[1] * 0.5
                    dur_mm = SCHED_D1[0] * bg * SCHED_FR[1] + SCHED_D1[1] * 0.25
                    dur_sq = SCHED_D1[0] * bg * fr_sq + SCHED_D1[1] * 0.25
                    if ITER_TIMES and (g, it) in ITER_TIMES:
                        t = ITER_TIMES[(g, it)]
                    events.append((t, 1, ("sx", g, it)))
                    for ci in range(nck):
                        events.append((t + dur_sx + dur_mm * (ci + 1) / nck,
                                       1, ("mm", g, it, ci)))
                    t += dur_sx + dur_mm + dur_sq
                    events.append((t, 1, ("sq", g, it)))
            events.sort(key=lambda ev: (ev[0], ev[1]))

            def next_inst_id():
                # consume one id as a marker; returns its numeric value
                return int(nc.get_next_instruction_name().split("-")[-1])

            EMIT_LOG.clear()
            for _, _, ev in events:
                lo = next_inst_id()
                if ev[0] == "chunk":
                    emit_chunk(ev[1], ev[2])
                elif ev[0] == "mm":
                    emit_mm(ev[1], ev[2], ev[3])
                elif ev[0] == "sx":
                    emit_sx(ev[1], ev[2])
                else:
                    emit_sq(ev[1], ev[2])
                EMIT_LOG.append((ev, lo, next_inst_id()))
    nc.compile()
    return nc


_NC = None


def kernel(x: np.ndarray, W: np.ndarray) -> np.ndarray:
    from concourse.bass_utils import run_bass_kernel_spmd

    global _NC
    if _NC is None:
        _NC = _build()

    x = np.ascontiguousarray(x, dtype=np.float32)
    w = np.ascontiguousarray(W.reshape(D, IK), dtype=np.float32)
    xs = x.reshape(NCORES, TOK, D)
    in_maps = [{"x": xs[i], "w": w} for i in range(NCORES)]
    res = run_bass_kernel_spmd(_NC, in_maps, core_ids=list(range(NCORES)))
    out = np.concatenate(
        [r["out"].reshape(BC, NCAP, DCAP) for r in res.results], axis=0
    )
    return out


if __name__ == "__main__":
    rng = np.random.default_rng(0)
    x = rng.standard_normal((B, S, D), dtype=np.float32)
    W = rng.standard_normal((1, D, IK), dtype=np.float32) * 0.1
    out = kernel(x, W)
    print(out.shape, out.dtype)


# revision 70
# speedup vs baseline: 1.0022x; 1.0008x over previous
"""Trainium2 Bass kernel for nn_Caps_Layer (capsule routing layer).

Reference computation (per batch b of 1024):
  u_hat[b] = (x[b] @ W).reshape(512, 5, 5) -> [5cap(i), 512(j), 5dim(k)]
  4 rounds of routing:
    c = softmax_i(blog); S[i,k] = sum_j c[i,j] u[i,j,k];
    o = S/||S||; blog[i,j] = sum_k o[i,k] u[i,j,k]
  output: o [1024, 5, 5]

Sharding: pure data parallel over batch across 8 cores (128 batches/core).

Per-core design (token-position on SBUF partitions; s = 4p + q):
  - x streamed in 8-batch gpsimd *casting* DMAs (f32 HBM -> fp16 SBUF,
    1920B contiguous runs): halves DMA time (~44us) and makes the whole
    downstream pipeline fp16.
  - PE transposes fp16 tiles at 1 cyc/row (the fp16 identity is the
    moving operand) [128t,120d] -> [120,128] fp16 PSUM (1 bank); PSUM->
    SBUF copies split DVE (fp16 2x mode) / ACT (f32-bitcast halves
    elems); GPSIMD cannot touch PSUM (walrus rule). fp16 GEMM vs fp16 W
    (1 cyc/row) -> u PSUM f32; W stored column-order (k,i) so the
    PSUM->u2 scatter merges (k,i) into one strided ACT copy per chunk.
  - u2 layout [128p, (k5, i5, q4, b)] fp16. Routing fully fp16 (1.5e-3
    global rel err vs 2e-2 budget); every DVE tensor op keeps innermost
    stride-1 fp16 APs to hit the 2x DVE mode. S-sums via PE ones-matmul
    (column sums replicated across partitions), q-accumulated in PSUM.
    k/i-plane sums use paired-plane tree adds (3 insts), or single
    X-axis reduces for the small tail groups (shorter chains).
  - 1/sqrt(ss) = exp(-0.5*ln(ss+eps)) on ACT; with the activation-table
    choice pinned to 'natural_log_exp_and_others' (index 6), Copy/Exp/Ln
    all share one table -> a single table load (saves ~47us of reloads).
  - uo products split DVE/Pool; fin on Pool -- balancing the elementwise
    engines (DVE is the pacing engine at ~78% occupancy).
  - Batches in groups of decreasing size [32,32,24,16,16,8]: early
    groups route while later x still streams; it0 S-matmuls are emitted
    per 16-batch routing chunk so they stream during phase 1; each
    routing iteration is emitted as three events (softmax / matmuls /
    squash) ordered by a tuned static time estimate so the in-order
    engine queues don't head-of-line block.
"""

import numpy as np

NCORES = 8
B, S, D = 1024, 512, 120
NCAP, DCAP = 5, 5
IK = NCAP * DCAP  # 25
BC = B // NCORES  # 128 batches per core
TOK = BC * S
EPS = 1e-7
ROUTINGS = 4

QB = 4                      # s-phases per partition (s = 4p + q)
CB = 8                      # batches per DMA chunk
NCHUNK = BC // CB           # 16
GROUPS = [32, 32, 24, 16, 16, 8]
CHUNK_RT = 16               # batches per routing psum chunk
UO_ON_DVE = {0, 5}          # groups whose uo-mul runs on DVE (rest: Pool)
XT_DVE_CHUNKS = 4           # first chunks: xt copies all on (idle) DVE
XT_MIX_CHUNKS = 4           # next chunks: alternate DVE/ACT
SCHED_CHUNK_T = 2.8         # est. us between chunk completions
SCHED_D0 = (0.10, 3.0)      # it0 duration estimate (a*bg + b)
SCHED_D1 = (0.18, 5.0)      # it1..3 duration estimate
SCHED_FR = (0.55, 0.30)     # sx/mm shares of an iter (sq gets the rest)
SCHED_IT0 = (0.4, 1.2)      # it0: mm offset after chunk; sq lag
ACT_TABLE_ID = 6            # natural_log_exp_and_others: copy+exp+ln

# Measured routing-iter completion times (us, from TimelineSim feedback);
# None -> use the analytic estimate. Keys: (g, it).
ITER_TIMES = None
EMIT_LOG = []               # [(tag, inst_lo, inst_hi)] filled during _build


def _patch_act_tables():
    """Pin the act-table chooser to set 6 (has copy+exp+ln together).

    bacc's insert_act_table_loads picks the FIRST table containing each
    activation's func, which alternates exp->set0 / ln->set5 and inserts a
    1283ns table load per switch. Presenting sets 0..5 as empty (names and
    positions preserved, so act_func_set_id stays a canonical act_info.json
    index) makes every func resolve to set 6 -> one load total.
    """
    import concourse.bacc as bacc

    if getattr(bacc, "_caps_tables_patched", False):
        return
    orig = bacc.get_activation_tables

    def patched(arch):
        tables = dict(orig(arch))
        names = list(tables.keys())
        out = {}
        for i, name in enumerate(names):
            out[name] = tables[name] if i == ACT_TABLE_ID else set()
        return out

    bacc.get_activation_tables = patched
    bacc._caps_tables_patched = True


def _build():
    import concourse.bass as bass
    import concourse.bacc as bacc
    import concourse.tile as tile
    from concourse import mybir
    from concourse.masks import make_identity

    _patch_act_tables()

    f32 = mybir.dt.float32
    f16 = mybir.dt.float16
    AF = mybir.ActivationFunctionType

    nc = bacc.Bacc("TRN2", target_bir_lowering=False, debug=False)
    x_d = nc.dram_tensor("x", [TOK, D], f32, kind="ExternalInput")
    w_d = nc.dram_tensor("w", [D, IK], f32, kind="ExternalInput")
    out_d = nc.dram_tensor("out", [1, BC * IK], f32, kind="ExternalOutput")

    # chunk c, partition p, free (bb, q, d); inner run (q d) = 480 f32
    # contiguous in HBM. token t = (c*CB + bb)*512 + 4p + q.
    x_all = x_d[:, :]

    def xv(c):
        return bass.AP(
            tensor=x_all.tensor,
            offset=x_all.offset + c * CB * S * D,
            ap=[[QB * D, 128], [S * D, CB], [1, QB * D]],
        )

    def ap_of(tile_ap, free_dims, extra_off=0, npart=None):
        part = list(tile_ap.ap[0])
        if npart is not None:
            part = [part[0], npart]
        return bass.AP(
            tensor=tile_ap.tensor,
            offset=tile_ap.offset + extra_off,
            ap=[part] + [list(d) for d in free_dims],
        )

    goff = [sum(GROUPS[:g]) for g in range(len(GROUPS))]

    with tile.TileContext(nc) as tc:
        with (
            tc.tile_pool(name="const", bufs=1) as const,
            tc.tile_pool(name="big", bufs=1) as big,
            tc.tile_pool(name="xin", bufs=12) as xin,
            tc.tile_pool(name="xtsb", bufs=4) as xtsb,
            tc.tile_pool(name="xtps", bufs=4, space="PSUM") as xtps,
            tc.tile_pool(name="ups", bufs=2, space="PSUM") as ups,
            tc.tile_pool(name="ops", bufs=2, space="PSUM") as ops_pool,
        ):
            # ---- constants ----
            w32 = const.tile([128, IK], f32)
            nc.sync.dma_start(out=w32[:D, :], in_=w_d[:, :])
            # w_sb column order (k, i): w_sb[:, k*5+i] = W[:, i*5+k]
            w_sb = const.tile([128, IK], f16)
            w_dst = ap_of(w_sb[:D, :], [[1, NCAP], [NCAP, DCAP]])
            w_src = ap_of(w32[:D, :], [[DCAP, NCAP], [1, DCAP]])
            nc.gpsimd.tensor_copy(out=w_dst, in_=w_src)
            ident = const.tile([128, 128], f16)
            make_identity(nc, ident[:])
            ones16 = const.tile([128, 128], f16)
            nc.vector.memset(ones16[:], 1.0)
            eps_t = const.tile([128, 1], f32)
            nc.vector.memset(eps_t[:], EPS)

            # ---- per-group persistent tensors (fp16 routing state) ----
            # u2/cu layout per group: (k5, i5, q4, b); strides b:1, q:Bg,
            # i:4Bg, k:20Bg
            u2 = [big.tile([128, DCAP, NCAP, QB, bg], f16, name=f"u2_{g}")
                  for g, bg in enumerate(GROUPS)]
            cu = [big.tile([128, DCAP, NCAP, QB, bg], f16, name=f"cu_{g}")
                  for g, bg in enumerate(GROUPS)]   # shared uo/cu scratch
            o2 = [big.tile([128, DCAP, NCAP, bg], f16, name=f"o2_{g}")
                  for g, bg in enumerate(GROUPS)]   # (k, i, b)
            blog = [big.tile([128, NCAP, QB, bg], f16, name=f"blog_{g}")
                    for g, bg in enumerate(GROUPS)]  # (i, q, b)
            scr1 = [big.tile([128, NCAP, QB, bg], f16, name=f"scr1_{g}")
                    for g, bg in enumerate(GROUPS)]
            e_t = [big.tile([128, NCAP, QB, bg], f16, name=f"e_{g}")
                   for g, bg in enumerate(GROUPS)]
            c_t = [big.tile([128, NCAP, QB, bg], f16, name=f"c_{g}")
                   for g, bg in enumerate(GROUPS)]
            z_t = [big.tile([128, QB, bg], f16, name=f"z_{g}")
                   for g, bg in enumerate(GROUPS)]  # (q, b)
            zi_t = [big.tile([128, QB, bg], f16, name=f"zi_{g}")
                    for g, bg in enumerate(GROUPS)]
            sq_t = [big.tile([128, DCAP, NCAP, bg], f16, name=f"sq_{g}")
                    for g, bg in enumerate(GROUPS)]  # (k, i, b)
            ss_t = [big.tile([128, NCAP, bg], f16, name=f"ss_{g}")
                    for g, bg in enumerate(GROUPS)]  # (i, b)
            ssl = [big.tile([128, NCAP, bg], f16, name=f"ssl_{g}")
                   for g, bg in enumerate(GROUPS)]   # ln(ss+eps)
            r_t = [big.tile([128, NCAP, bg], f16, name=f"r_{g}")
                   for g, bg in enumerate(GROUPS)]
            fin = [big.tile([1, bg * IK], f32, name=f"fin_{g}")
                   for g, bg in enumerate(GROUPS)]

            # ================= Phase 1: one 4-batch chunk =================
            def emit_chunk(g, c):
                """DMA 8 batches, transpose, GEMM, scatter into u2[g]."""
                bg = GROUPS[g]
                b_in_g = c * CB - goff[g]  # first batch idx within group
                x4 = xin.tile([128, CB * QB * D], f16, name="x4")
                nc.gpsimd.dma_start(out=x4[:], in_=xv(c))  # casting DMA
                for half in range(2):
                    u_ps = ups.tile([128, 4 * QB * IK], f32, name="u_ps")
                    for hh in range(2):
                        xt_ps = xtps.tile([128, 1024], f16, name="xt_ps")
                        xt_sb = xtsb.tile([128, 1024], f16, name="xt_sb")
                        for bb in range(2):
                            boff = (half * 2 + hh) * 2 + bb
                            for q in range(QB):
                                t = bb * QB + q
                                src = x4[:, boff * QB * D + q * D:
                                         boff * QB * D + (q + 1) * D]
                                nc.tensor.transpose(
                                    xt_ps[:D, t * 128:(t + 1) * 128], src,
                                    ident[:],
                                )
                        # xt copy PSUM->SBUF: GPSIMD can't read PSUM, so
                        # only ACT (f32-bitcast halves elems) or DVE
                        # (fp16 2x mode) are legal. Early chunks go to DVE
                        # (idle until routing ramps up), later ones to ACT.
                        if c < XT_DVE_CHUNKS:
                            dve_copy = True
                        elif c < XT_DVE_CHUNKS + XT_MIX_CHUNKS:
                            dve_copy = (2 * half + hh) % 2 == 0
                        else:
                            dve_copy = False
                        if dve_copy:
                            nc.vector.tensor_copy(out=xt_sb[:D, :],
                                                  in_=xt_ps[:D, :])
                        else:
                            nc.scalar.copy(out=xt_sb[:D, :].bitcast(f32),
                                           in_=xt_ps[:D, :].bitcast(f32))
                        for t in range(8):
                            tt = hh * 8 + t
                            nc.tensor.matmul(
                                u_ps[:, tt * IK:(tt + 1) * IK],
                                xt_sb[:D, t * 128:(t + 1) * 128],
                                w_sb[:D, :],
                                start=True, stop=True,
                            )
                    # scatter psum (hh,bb,q,(k,i)) -> u2[g] (k,i,q,b): one
                    # copy; (k,i) merges on dst (k stride 20bg = 5 * 4bg)
                    src = ap_of(u_ps[:], [[100, 4], [25, 4], [1, 25]])
                    dst = ap_of(u2[g][:], [[1, 4], [bg, 4], [4 * bg, 25]],
                                extra_off=b_in_g + half * 4)
                    nc.scalar.copy(out=dst, in_=src)

            # ================= Phase 2: routing =================
            def emit_sx(g, it):
                """Softmax part of iter it>=1: uo -> blog -> exp -> c -> cu."""
                bg = GROUPS[g]
                u2g, cug, o2g = u2[g][:], cu[g][:], o2[g][:]
                small = bg <= 16   # short-chain variant for tail groups
                PL = 20 * bg   # (i,q,b) plane elems
                IPL = 4 * bg   # (q,b) plane elems
                # uo = u2 * o2_bcast(q); iter ((k,i),(q),(b))
                o2_bc = ap_of(o2g, [[bg, IK], [0, QB], [1, bg]])
                u2_v = ap_of(u2g, [[4 * bg, IK], [bg, QB], [1, bg]])
                cu_v = ap_of(cug, [[4 * bg, IK], [bg, QB], [1, bg]])
                eng = nc.vector if g in UO_ON_DVE else nc.gpsimd
                eng.tensor_mul(cu_v, u2_v, o2_bc)
                bl, s1 = blog[g][:], scr1[g][:]
                if small:
                    # blog_raw = sum_k uo in one X-reduce over k
                    uo_r = ap_of(cug, [[1, PL], [PL, DCAP]])
                    with nc.allow_low_precision(reason="fp16 5-term sum"):
                        nc.vector.reduce_sum(
                            bl, uo_r, axis=mybir.AxisListType.X)
                else:
                    # blog_raw = sum_k uo; paired-plane adds (3 insts):
                    # uo[{0,1}] <- uo[{0,2}] + uo[{1,3}] in one op (safe:
                    # writes trail reads in element order), then combine.
                    uo_k = lambda k: ap_of(cug, [[1, PL]],
                                           extra_off=k * PL)
                    pair_a = ap_of(cug, [[2 * PL, 2], [1, PL]])
                    pair_b = ap_of(cug, [[2 * PL, 2], [1, PL]],
                                   extra_off=PL)
                    s_pairs = ap_of(cug, [[1, 2 * PL]])
                    nc.vector.tensor_add(s_pairs, pair_a, pair_b)
                    nc.vector.tensor_add(bl, uo_k(0), uo_k(1))
                    nc.vector.tensor_add(bl, bl, uo_k(4))
                # temperature: blog *= r_bcast(q)  (r from prev iter)
                r_bc = ap_of(r_t[g][:], [[bg, NCAP], [0, QB], [1, bg]])
                nc.vector.tensor_mul(bl, bl, r_bc)
                # softmax over i
                nc.scalar.activation(out=e_t[g][:], in_=bl, func=AF.Exp)
                zt, zit = z_t[g][:], zi_t[g][:]
                if small:
                    # z = sum_i e in one X-reduce over i (outer-stride)
                    e_r = ap_of(e_t[g][:], [[1, IPL], [IPL, NCAP]])
                    with nc.allow_low_precision(reason="fp16 5-term sum"):
                        nc.vector.reduce_sum(
                            zt, e_r, axis=mybir.AxisListType.X)
                else:
                    # z tree with one paired-plane add into scr1
                    e_i = lambda i: ap_of(e_t[g][:], [[1, IPL]],
                                          extra_off=i * IPL)
                    zp_a = ap_of(e_t[g][:], [[2 * IPL, 2], [1, IPL]])
                    zp_b = ap_of(e_t[g][:], [[2 * IPL, 2], [1, IPL]],
                                 extra_off=IPL)
                    zp_out = ap_of(s1, [[1, 2 * IPL]])
                    nc.vector.tensor_add(zp_out, zp_a, zp_b)
                    nc.vector.tensor_add(zt, ap_of(s1, [[1, IPL]]),
                                         ap_of(s1, [[1, IPL]],
                                               extra_off=IPL))
                    nc.vector.tensor_add(zt, zt, e_i(4))
                with nc.allow_low_precision(reason="fp16 ok at 2e-2 tol"):
                    nc.vector.reciprocal(zit, zt)
                zi_bc = ap_of(zit, [[0, NCAP], [1, IPL]])
                nc.vector.tensor_mul(c_t[g][:], e_t[g][:], zi_bc)
                # cu = u2 * c_bcast(k)
                c_bc = ap_of(c_t[g][:], [[0, DCAP], [1, PL]])
                nc.vector.tensor_mul(cug, u2g, c_bc)

            def emit_mm(g, it, ci):
                """S-matmuls + o2 copy for one 16-batch routing chunk."""
                bg = GROUPS[g]
                KPL = NCAP * bg
                src_t = u2[g][:] if it == 0 else cu[g][:]
                b0 = ci * CHUNK_RT
                cb = min(CHUNK_RT, bg - b0)
                o_ps = ops_pool.tile([128, CHUNK_RT * IK], f32, name="o_ps")
                for q in range(QB):
                    rhs = ap_of(
                        src_t, [[1, cb], [4 * bg, NCAP], [20 * bg, DCAP]],
                        extra_off=q * bg + b0)
                    nc.tensor.matmul(
                        o_ps[:, :cb * IK], ones16[:], rhs,
                        start=(q == 0), stop=(q == QB - 1),
                    )
                # psum (b,i,k) -> o2 (k,i,b), fp16 cast. Tail groups copy
                # on DVE so copy->sq->ss runs on one engine (fewer chain
                # hops); big groups stay on ACT for busy-balance.
                dst = ap_of(o2[g][:], [[1, cb], [bg, NCAP], [KPL, DCAP]],
                            extra_off=b0)
                if bg <= CHUNK_RT and it == ROUTINGS - 1:
                    nc.vector.tensor_copy(out=dst, in_=o_ps[:, :cb * IK])
                else:
                    nc.scalar.copy(out=dst, in_=o_ps[:, :cb * IK])

            def emit_sq(g, it):
                """Squash stats: r = exp(-0.5*ln(ss+eps)); ss = sum_k o2^2."""
                bg = GROUPS[g]
                o2g = o2[g][:]
                small = bg <= 16
                KPL = NCAP * bg
                last = it == ROUTINGS - 1
                sqg, ssg = sq_t[g][:], ss_t[g][:]
                nc.vector.tensor_mul(sqg, o2g, o2g)
                if small:
                    sq_r = ap_of(sqg, [[1, KPL], [KPL, DCAP]])
                    with nc.allow_low_precision(reason="fp16 5-term sum"):
                        nc.vector.reduce_sum(
                            ssg, sq_r, axis=mybir.AxisListType.X)
                else:
                    # ss tree with one paired-plane add into sq[{0,1}]
                    sq_k = lambda k: ap_of(sqg, [[1, KPL]], extra_off=k * KPL)
                    sp_a = ap_of(sqg, [[2 * KPL, 2], [1, KPL]])
                    sp_b = ap_of(sqg, [[2 * KPL, 2], [1, KPL]],
                                 extra_off=KPL)
                    sp_out = ap_of(sqg, [[1, 2 * KPL]])
                    nc.vector.tensor_add(sp_out, sp_a, sp_b)
                    nc.vector.tensor_add(ssg, sq_k(0), sq_k(1))
                    nc.vector.tensor_add(ssg, ssg, sq_k(4))
                nc.scalar.activation(
                    out=ssl[g][:], in_=ssg, func=AF.Ln, bias=eps_t[:])
                nc.scalar.activation(
                    out=r_t[g][:], in_=ssl[g][:], func=AF.Exp, scale=-0.5)
                if last:
                    # fin[(b,i,k)] = o2[(k,i,b)] * r, row 0 (rows equal), f32
                    o2_row = ap_of(o2g, [[1, bg], [bg, NCAP], [KPL, DCAP]],
                                   npart=1)
                    r_row = ap_of(r_t[g][:], [[1, bg], [bg, NCAP], [0, DCAP]],
                                  npart=1)
                    fin_v = ap_of(fin[g][:], [[IK, bg], [DCAP, NCAP], [1, DCAP]])
                    # tail groups: fin on DVE (faster + no Pool queue hop
                    # at the very end of the terminal chain)
                    fin_eng = nc.vector if bg <= CHUNK_RT else nc.gpsimd
                    fin_eng.tensor_mul(fin_v, o2_row, r_row)
                    nc.sync.dma_start(
                        out=out_d[:, goff[g] * IK:(goff[g] + bg) * IK],
                        in_=fin[g][:],
                    )

            # ---- static emission schedule (time-ordered) ----
            CHUNK_T = SCHED_CHUNK_T
            events = []
            for g, bg in enumerate(GROUPS):
                c0 = goff[g] // CB
                nch = bg // CB
                for j in range(nch):
                    events.append(((c0 + j + 1) * CHUNK_T - 1e-6, 0,
                                   ("chunk", g, c0 + j)))
            for g, bg in enumerate(GROUPS):
                nck = (bg + CHUNK_RT - 1) // CHUNK_RT
                # it0 S-matmul chunks stream as their phase chunks land
                t = 0.0
                for ci in range(nck):
                    hi_b = goff[g] + min((ci + 1) * CHUNK_RT, bg)
                    cc = (hi_b + CB - 1) // CB   # covering phase chunk count
                    t = cc * CHUNK_T + SCHED_IT0[0]
                    events.append((t, 1, ("mm", g, 0, ci)))
                t += SCHED_IT0[1] + 0.01 * bg
                events.append((t, 1, ("sq", g, 0)))
                for it in range(1, ROUTINGS):
                    fr_sq = 1.0 - SCHED_FR[0] - SCHED_FR[1]
                    dur_sx = SCHED_D1[0] * bg * SCHED_FR[0] + SCHED_D1[1] * 0.5
                    dur_mm = SCHED_D1[0] * bg * SCHED_FR[1] + SCHED_D1[1] * 0.25
                    dur_sq = SCHED_D1[0] * bg * fr_sq + SCHED_D1[1] * 0.25
                    if ITER_TIMES and (g, it) in ITER_TIMES:
                        t = ITER_TIMES[(g, it)]
                    events.append((t, 1, ("sx", g, it)))
                    for ci in range(nck):
                        events.append((t + dur_sx + dur_mm * (ci + 1) / nck,
                                       1, ("mm", g, it, ci)))
                    t += dur_sx + dur_mm + dur_sq
                    events.append((t, 1, ("sq", g, it)))
            events.sort(key=lambda ev: (ev[0], ev[1]))

            def next_inst_id():
                # consume one id as a marker; returns its numeric value
                return int(nc.get_next_instruction_name().split("-")[-1])

            EMIT_LOG.clear()
            for _, _, ev in events:
                lo = next_inst_id()
                if ev[0] == "chunk":
                    emit_chunk(ev[1], ev[2])
                elif ev[0] == "mm":
                    emit_mm(ev[1], ev[2], ev[3])
                elif ev[0] == "sx":
                    emit_sx(ev[1], ev[2])
                else:
                    emit_sq(ev[1], ev[2])
                EMIT_LOG.append((ev, lo, next_inst_id()))
    nc.compile()
    return nc


_NC = None


def kernel(x: np.ndarray, W: np.ndarray) -> np.ndarray:
    from concourse.bass_utils import run_bass_kernel_spmd

    global _NC
    if _NC is None:
        _NC = _build()

    x = np.ascontiguousarray(x, dtype=np.float32)
    w = np.ascontiguousarray(W.reshape(D, IK), dtype=np.float32)
    xs = x.reshape(NCORES, TOK, D)
    in_maps = [{"x": xs[i], "w": w} for i in range(NCORES)]
    res = run_bass_kernel_spmd(_NC, in_maps, core_ids=list(range(NCORES)))
    out = np.concatenate(
        [r["out"].reshape(BC, NCAP, DCAP) for r in res.results], axis=0
    )
    return out


if __name__ == "__main__":
    rng = np.random.default_rng(0)
    x = rng.standard_normal((B, S, D), dtype=np.float32)
    W = rng.standard_normal((1, D, IK), dtype=np.float32) * 0.1
    out = kernel(x, W)
    print(out.shape, out.dtype)
